# revision 5
# baseline (speedup 1.0000x reference)
"""BCQLinear (3-bit binary-coded quantized linear) Trainium2 kernel.

Full-input contract: kernel(**inputs) takes the unsharded inputs of
nn_BCQLinear_88510686036218 and returns the full [1, 128, 4096] output.

Math: w = alpha*(2*S-7) + beta with S in [0,8) the 3-bit code, then
y = (x[:, in_reorder] @ w)[:, out_reorder].
Split: device computes y_q = x @ (2*alpha*S); host adds the rank-32
beta part corr = xsum.T @ (beta - 7*alpha) (0.01% of the FLOPs).

Sharding: out-features split 8 ways (512 cols/core), x replicated.

Per-core device program (SPMD, one Bass program):
  - Contraction rows are band-packed: K-tile kt in [0,32), partition p:
    dequant row i(kt,p) = 128*(p//4) + 4*kt + (p%4), so a single [128,512]
    alpha tile (a2p[p,:] ~ alpha[p//4,:]) serves every K-tile.
  - Codes arrive as packed int32 words with 4 3-bit fields per int16
    half (field r at bits [3r,3r+3)); code (j, r) maps to out-col
    o' = r*128 + j.  Dequant is two DVE passes per chunk:
      extract: vt = W32 & ((7<<3r) in both halves), one TensorScalar
               per r; leaves the code scaled by 8^r, which rides
               through the matmul and is divided out on the host.
      scale:   wm[p,kt,o'] = vt[p,r,kt,u] * a2p[p,o'] -- one op per
               chunk writing the k-major layout the matmuls stream.
  - ~30 garbage warm-up matmuls at body start keep the PE busy through
    the HAM activity window so the real matmul stream runs at 2.4 GHz
    (otherwise every matmul pays the cold 1.2 GHz clock gate).
  - Chunked pipeline [2,6,8,8,8] K-tiles: the small first chunk lands
    early so dequant/matmuls start sooner.
  - Matmuls: stationary xt[128, T] per K-tile, streaming wm halves
    [128, 256] into two PSUM banks (fp16 operands, fp32 PSUM).
  - All DMAs ride the two HWDGE queues (sync: weight chunks, scalar:
    a2p + xt); the SWDGE/gpsimd queue stays idle.  No ACT-engine ops,
    so no 1.3us ACT_TABLE_LOAD on the scalar queue before its DMAs.
  - The last chunk runs h0 (fields r=0,1) first: its psum copy + DMA
    overlap the h1 matmul burst.
"""
import numpy as np
from contextlib import ExitStack

import concourse.bass as bass
import concourse.mybir as mybir
import concourse.tile as tile
from concourse import bacc

IN_F, OUT_F, WBITS, GS, OFI = 4096, 4096, 3, 128, 128
NG, NB = 32, 32
NCORES = 8
OPC = OUT_F // NCORES        # 512 out-cols per core
NKT = 32                     # K-tiles of 128 rows
NR = 4                       # fields per int16 half
NWC = OPC // (2 * NR)        # 64 packed words per (partition, K-tile)
T = 128                      # tokens
KT_CHUNKS = [2, 6, 8, 8, 8]  # pipeline chunk sizes (K-tiles)
NCHUNK = len(KT_CHUNKS)
KT_OFF = [sum(KT_CHUNKS[:i]) for i in range(NCHUNK)]
N_WARM = 30                  # PE warm-up matmuls (HAM un-throttle)
SCALE_STT = False            # scale via per-r scalar_tensor_tensor (4x cand.)

F32 = mybir.dt.float32
F16 = mybir.dt.float16
I32 = mybir.dt.int32
I16 = mybir.dt.int16
ALU = mybir.AluOpType

_PROGRAM_CACHE = {}


# ---------------------------------------------------------------- host prep
def _dequant_codes(qweight):
    """S[i, o] in [0,8): w = alpha*(2S-7)+beta."""
    qw = np.asarray(qweight, dtype=np.uint32).reshape(NG, NB, WBITS, GS * OFI // 32)
    bits = (qw[..., None] >> np.arange(32, dtype=np.uint32)) & 1
    bits = bits.reshape(NG, NB, WBITS, GS, OFI)
    S = (bits * (2 ** np.arange(WBITS, dtype=np.uint32))[:, None, None]).sum(axis=2)
    return S.transpose(0, 2, 1, 3).reshape(IN_F, OUT_F).astype(np.uint32)


def _band_rows():
    kt, p = np.meshgrid(np.arange(NKT), np.arange(128), indexing="ij")
    return 128 * (p // 4) + 4 * kt + (p % 4)      # [NKT, 128]


def _prepare(inputs):
    x = np.asarray(inputs["x"], np.float32).reshape(-1, IN_F)
    alpha = np.asarray(inputs["alpha"], np.float32)
    beta = np.asarray(inputs["beta"], np.float32)
    in_reorder = np.asarray(inputs["in_reorder"], np.int64)
    xf = x[:, in_reorder]

    S = _dequant_codes(inputs["qweight"])          # [IN_F, OUT_F] uint32
    rows = _band_rows()                            # [NKT, 128]
    XT = np.ascontiguousarray(
        xf[:, rows.reshape(-1)].T.reshape(NKT, 128, T).transpose(1, 0, 2)
    ).reshape(128, NKT * T).astype(np.float16)     # [p, kt*T]

    # host-side beta part: corr[t, o] = sum_g xsum[g,t] * (beta-7a)[g,o]
    xsum = xf.reshape(T, NG, GS).sum(axis=2, dtype=np.float64)   # [T, NG]
    Bfull = beta.astype(np.float64) - 7.0 * alpha.astype(np.float64)
    corr = (xsum @ Bfull).astype(np.float32)       # [T, OUT_F]

    in_maps = []
    for c in range(NCORES):
        cols = slice(OPC * c, OPC * (c + 1))
        # codes for this core in banded row order: [p, kt, o']
        Sc = S[rows.reshape(-1), cols].reshape(NKT, 128, OPC).transpose(1, 0, 2)
        # pack fields: o' = r*128 + 2c' + h  ->  bits [3r+16h, +3)
        W = np.zeros((128, NKT, NWC), np.uint32)
        for r in range(NR):
            for h in range(2):
                W |= Sc[:, :, r * 128 + h::2][:, :, :NWC] << (3 * r + 16 * h)
        W = W.reshape(128, NKT * NWC)
        a2p = (2.0 * alpha[np.arange(128) // 4][:, cols]).astype(np.float16)
        im = dict(a2p=a2p)
        for ch in range(NCHUNK):
            k0, nk = KT_OFF[ch], KT_CHUNKS[ch]
            im[f"w{ch}"] = np.ascontiguousarray(
                W[:, k0 * NWC:(k0 + nk) * NWC]).view(np.int32)
        for ch in range(NCHUNK):
            k0, nk = KT_OFF[ch], KT_CHUNKS[ch]
            im[f"xt{ch}"] = np.ascontiguousarray(XT[:, k0 * T:(k0 + nk) * T])
        in_maps.append(im)
    return in_maps, corr


# ---------------------------------------------------------------- program
def build_program():
    nc = bacc.Bacc("TRN2")

    w_dr = [nc.declare_dram_parameter(f"w{ch}", [128, KT_CHUNKS[ch] * NWC], I32,
                                      isOutput=False)
            for ch in range(NCHUNK)]
    xt_dr = [nc.declare_dram_parameter(f"xt{ch}", [128, KT_CHUNKS[ch] * T], F16,
                                       isOutput=False)
             for ch in range(NCHUNK)]
    a2p_dr = nc.declare_dram_parameter("a2p", [128, OPC], F16, isOutput=False)
    z = nc.declare_dram_parameter("z", [T, OPC], F16, isOutput=True)

    with tile.TileContext(nc) as tc, ExitStack() as ctx:
        cpool = ctx.enter_context(tc.tile_pool(name="const", bufs=1))
        opool = ctx.enter_context(tc.tile_pool(name="out", bufs=1))
        ppool = ctx.enter_context(tc.tile_pool(name="psum", bufs=1, space="PSUM"))

        # --- static tiles --------------------------------------------------
        w_sb = [cpool.tile([128, KT_CHUNKS[ch] * NWC], I32, tag=f"w{ch}",
                           name=f"wsb{ch}")
                for ch in range(NCHUNK)]
        xt_sb = [cpool.tile([128, KT_CHUNKS[ch] * T], F16, tag=f"xt{ch}",
                            name=f"xtsb{ch}")
                 for ch in range(NCHUNK)]
        a2_sb = cpool.tile([128, OPC], F16, tag="a2")
        # extracted codes (r-major) and dequantized weights (k-major)
        vt = [cpool.tile([128, KT_CHUNKS[ch] * OPC], I16, tag=f"vt{ch}",
                         name=f"vtt{ch}")
              for ch in range(NCHUNK)]
        wm = [cpool.tile([128, KT_CHUNKS[ch] * OPC], F16, tag=f"wm{ch}",
                         name=f"wmt{ch}")
              for ch in range(NCHUNK)]
        # PE warm-up operands (zeros; values irrelevant)
        gw = cpool.tile([128, 128], F16, tag="gw")

        HALF = OPC // 2
        psum_h = [ppool.tile([T, HALF], F32, tag=f"main{h}", name=f"psum{h}")
                  for h in range(2)]
        psum_scr = ppool.tile([128, 128], F32, tag="scr")

        # --- DMA schedule (issued first so the queues start immediately) --
        nc.sync.dma_start(out=w_sb[0][:], in_=w_dr[0][:])
        nc.scalar.dma_start(out=a2_sb[:], in_=a2p_dr[:])
        nc.scalar.dma_start(out=xt_sb[0][:], in_=xt_dr[0][:])
        nc.sync.dma_start(out=w_sb[1][:], in_=w_dr[1][:])
        nc.scalar.dma_start(out=xt_sb[1][:], in_=xt_dr[1][:])
        nc.sync.dma_start(out=w_sb[2][:], in_=w_dr[2][:])
        nc.scalar.dma_start(out=xt_sb[2][:], in_=xt_dr[2][:])
        nc.sync.dma_start(out=w_sb[3][:], in_=w_dr[3][:])
        nc.scalar.dma_start(out=xt_sb[3][:], in_=xt_dr[3][:])
        nc.sync.dma_start(out=w_sb[4][:], in_=w_dr[4][:])
        nc.scalar.dma_start(out=xt_sb[4][:], in_=xt_dr[4][:])

        # --- PE warm-up: keep the array busy through the HAM window -------
        nc.gpsimd.memset(gw[:], 0.0)
        for _ in range(N_WARM):
            nc.tensor.matmul(psum_scr[:], gw[:], gw[:], start=True, stop=True)

        # --- dequant pass 1: vt[p, r, kt, u] = W32 & (7<<3r | ..<<16) -----
        def extract(ch, r):
            nk = KT_CHUNKS[ch]
            m = 7 << (3 * r)
            nc.vector.tensor_scalar(
                vt[ch][:, r * nk * 128:(r + 1) * nk * 128].bitcast(I32),
                w_sb[ch][:], (m << 16) | m, None, ALU.bitwise_and)

        # --- dequant pass 2: wm[p, kt, r*128+u] = vt * a2p ----------------
        def scale(ch, h):
            # h None: all four r-blocks in one op; else the two of half h
            nk = KT_CHUNKS[ch]
            r0, nr = (0, NR) if h is None else (2 * h, 2)
            if SCALE_STT:
                # STT is limited to 3D APs: one op per r-block
                for r in range(r0, r0 + nr):
                    in0 = vt[ch][:, r * nk * 128:(r + 1) * nk * 128].rearrange(
                        "p (k u) -> p k u", u=128)
                    out = wm[ch][:].rearrange(
                        "p (k o) -> p k o", o=OPC)[:, :, r * 128:(r + 1) * 128]
                    in1 = a2_sb[:, r * 128:(r + 1) * 128].unsqueeze(
                        1).broadcast_to([128, nk, 128])
                    nc.vector.scalar_tensor_tensor(
                        out, in0, 0.0, in1, ALU.add, ALU.mult)
            else:
                in0 = vt[ch][:].rearrange("p (r k u) -> p r k u", r=NR,
                                          u=128)[:, r0:r0 + nr]
                out = wm[ch][:].rearrange("p (k r u) -> p r k u", r=NR,
                                          u=128)[:, r0:r0 + nr]
                in1 = a2_sb[:, r0 * 128:(r0 + nr) * 128].rearrange(
                    "p (r u) -> p r u", r=nr).unsqueeze(2).broadcast_to(
                    [128, nr, nk, 128])
                nc.vector.tensor_tensor(out, in0, in1, ALU.mult)

        def mms(ch, h):
            k0, nk = KT_OFF[ch], KT_CHUNKS[ch]
            for kl in range(nk):
                kt = k0 + kl
                nc.tensor.matmul(
                    psum_h[h][:],
                    xt_sb[ch][:, kl * T:(kl + 1) * T],
                    wm[ch][:, kl * OPC + h * HALF:kl * OPC + (h + 1) * HALF],
                    start=(kt == 0),
                    stop=(kt == NKT - 1),
                )

        last = NCHUNK - 1
        for ch in range(last):
            for r in range(NR):
                extract(ch, r)
            scale(ch, None)
            mms(ch, 0)
            mms(ch, 1)
        # last chunk: h0 fields+matmuls first; h0 copy/DMA overlap h1 burst
        out_a = opool.tile([T, HALF], F16, tag="out_a")
        out_b = opool.tile([T, HALF], F16, tag="out_b")
        extract(last, 0)
        extract(last, 1)
        scale(last, 0)
        mms(last, 0)
        extract(last, 2)
        extract(last, 3)
        scale(last, 1)
        nc.vector.tensor_copy(out_a[:], psum_h[0][:])
        nc.scalar.dma_start(out=z[:, :HALF], in_=out_a[:])
        mms(last, 1)
        nc.vector.tensor_copy(out_b[:], psum_h[1][:])
        nc.sync.dma_start(out=z[:, HALF:], in_=out_b[:])
    nc.finalize()
    return nc


def _get_program():
    if "nc" not in _PROGRAM_CACHE:
        _PROGRAM_CACHE["nc"] = build_program()
    return _PROGRAM_CACHE["nc"]


# ---------------------------------------------------------------- entry
def kernel(**inputs):
    from concourse.bass_utils import run_bass_kernel_spmd

    in_maps, corr = _prepare(inputs)
    nc = _get_program()
    res = run_bass_kernel_spmd(nc, in_maps, list(range(NCORES)))
    zf = np.concatenate(
        [res.results[c]["z"].astype(np.float32) for c in range(NCORES)], axis=1)
    rs = np.tile(np.repeat(8.0 ** -np.arange(NR), 2 * NWC), NCORES)
    out_reorder = np.asarray(inputs["out_reorder"], np.int64)
    y = (zf * rs[None, :] + corr)[:, out_reorder]
    return y.reshape(1, T, OUT_F).astype(np.float32)


# revision 7
# speedup vs baseline: 1.2468x; 1.2468x over previous
"""BCQLinear (3-bit binary-coded quantized linear) Trainium2 kernel.

Full-input contract: kernel(**inputs) takes the unsharded inputs of
nn_BCQLinear_88510686036218 and returns the full [1, 128, 4096] output.

Math: w = alpha*(2*S-7) + beta with S in [0,8) the 3-bit code, then
y = (x[:, in_reorder] @ w)[:, out_reorder].

Sharding: out-features split 8 ways (512 cols/core), x replicated.

Hybrid weight path (the key idea): the DVE dequant (bit-extract +
alpha-multiply) runs at ~0.5 ns/element and is the kernel's critical
resource, while the DMA fabric has slack.  So each core's 512 columns
split:
  - PACKED 320 cols (local 0..319): 3-bit codes packed 4-per-int16
    (field r at bits [3r,3r+3), col o' = r*80+u), dequantized on-device:
      extract: vt = W32 & mask(r), one int32 TensorScalar per (chunk,r)
      scale:   wm[p,kt,o'] = vt * a2p, TensorTensor per ~4-5 K-tiles
    The 8^r field scale rides through the matmul, divided out on host;
    the beta part is a host-side rank-32 correction (corr).
  - DIRECT 192 cols (local 320..511): the host dequantizes fully
    (alpha*(2S-7)+beta, exact in fp16) and streams fp16 weights; no
    DVE work, no beta correction.

DMA discipline (what actually matters on this fabric): the 16 SDMA
engines round-robin between queues at PACKET granularity, so byte
throughput is proportional to descriptor (= dram-row) size; small-row
transfers starve next to big-row ones, and a queue's transfers complete
in FIFO order.  Hence:
  - [a2p | w0] are merged into one early small transfer (the DVE's
    critical input), [xt_c | wd_c] are merged per chunk into one
    "mega" with uniform multi-KB rows, and the w_c extracts ride their
    own small transfers ordered ahead of the megas they beat.
  - chunks [4,10,12,6]: first chunk small (dequant starts ~2.5us in),
    last chunk small (its mega lands last; short matmul tail).

Other schedule points:
  - Contraction rows band-packed: row i(kt,p) = 128*(p//4)+4*kt+(p%4),
    so one [128,320] alpha tile serves every K-tile.
  - ~16 garbage warm-up matmuls bridge the DMA head so the PE is busy
    through the HAM activity window (real matmuls then run at 2.4 GHz,
    not the cold 1.2 GHz clock gate).
  - Per chunk: A-matmuls (packed, gated on DVE scale pieces) are
    emitted before B-matmuls (direct, gated on the chunk's mega DMA),
    matching their expected ready times (PE executes in order).
  - psA/psB are padded to full 2KB PSUM banks so PE writes and DVE
    reads never share a bank (collision is fatal).
  - No ACT-engine ops (avoids the 1.3us ACT_TABLE_LOAD on the scalar
    queue); output copies on DVE.
"""
import numpy as np
from contextlib import ExitStack

import concourse.bass as bass
import concourse.mybir as mybir
import concourse.tile as tile
from concourse import bacc

IN_F, OUT_F, WBITS, GS, OFI = 4096, 4096, 3, 128, 128
NG, NB = 32, 32
NCORES = 8
OPC = OUT_F // NCORES        # 512 out-cols per core
NPK = 320                    # packed cols per core (local 0..NPK)
NDIR = OPC - NPK             # direct fp16 cols per core
WPF = NPK // 4               # cols per field r (80)
NWC = NPK // 8               # packed int32 words per (partition, K-tile)
NKT = 32                     # K-tiles of 128 rows
NR = 4                       # fields per int16 half
T = 128                      # tokens
KT_CHUNKS = [4, 10, 12, 6]   # DMA pipeline chunk sizes (K-tiles)
NCHUNK = len(KT_CHUNKS)
KT_OFF = [sum(KT_CHUNKS[:i]) for i in range(NCHUNK)]
SCALE_PIECES = {0: [4], 1: [5, 5], 2: [4, 4, 4], 3: [3, 3]}
N_WARM = 16                  # PE warm-up matmuls (HAM un-throttle)

F32 = mybir.dt.float32
F16 = mybir.dt.float16
I32 = mybir.dt.int32
I16 = mybir.dt.int16
ALU = mybir.AluOpType

_PROGRAM_CACHE = {}


# ---------------------------------------------------------------- host prep
def _dequant_codes(qweight):
    """S[i, o] in [0,8): w = alpha*(2S-7)+beta."""
    qw = np.asarray(qweight, dtype=np.uint32).reshape(NG, NB, WBITS, GS * OFI // 32)
    bits = (qw[..., None] >> np.arange(32, dtype=np.uint32)) & 1
    bits = bits.reshape(NG, NB, WBITS, GS, OFI)
    S = (bits * (2 ** np.arange(WBITS, dtype=np.uint32))[:, None, None]).sum(axis=2)
    return S.transpose(0, 2, 1, 3).reshape(IN_F, OUT_F).astype(np.uint32)


def _band_rows():
    kt, p = np.meshgrid(np.arange(NKT), np.arange(128), indexing="ij")
    return 128 * (p // 4) + 4 * kt + (p % 4)      # [NKT, 128]


def _prepare(inputs):
    x = np.asarray(inputs["x"], np.float32).reshape(-1, IN_F)
    alpha = np.asarray(inputs["alpha"], np.float32)
    beta = np.asarray(inputs["beta"], np.float32)
    in_reorder = np.asarray(inputs["in_reorder"], np.int64)
    xf = x[:, in_reorder]

    S = _dequant_codes(inputs["qweight"])          # [IN_F, OUT_F] uint32
    rows = _band_rows()                            # [NKT, 128]
    rowsf = rows.reshape(-1)
    XT = np.ascontiguousarray(
        xf[:, rowsf].T.reshape(NKT, 128, T).transpose(1, 0, 2)
    ).reshape(128, NKT * T).astype(np.float16)     # [p, kt*T]

    # host-side beta part (packed cols only):
    # corr[t, o] = sum_g xsum[g,t] * (beta-7a)[g,o]
    xsum = xf.reshape(T, NG, GS).sum(axis=2, dtype=np.float64)   # [T, NG]
    Bfull = beta.astype(np.float64) - 7.0 * alpha.astype(np.float64)
    corr = (xsum @ Bfull).astype(np.float32)       # [T, OUT_F]

    g_of_row = rowsf // GS                         # group of each banded row

    in_maps = []
    for c in range(NCORES):
        pk = slice(OPC * c, OPC * c + NPK)         # packed global cols
        dr = slice(OPC * c + NPK, OPC * (c + 1))   # direct global cols
        # packed codes, banded: [p, kt, o']
        Sc = S[rowsf, pk].reshape(NKT, 128, NPK).transpose(1, 0, 2)
        W = np.zeros((128, NKT, NWC), np.uint32)
        for r in range(NR):
            for h in range(2):
                W |= Sc[:, :, r * WPF + h::2][:, :, :NWC] << (3 * r + 16 * h)
        W = W.reshape(128, NKT * NWC)
        a2p = (2.0 * alpha[np.arange(128) // 4][:, pk]).astype(np.float16)
        # direct fp16 weights, banded k-major: wd[p, kt*NDIR + q]
        Sd = S[rowsf, dr].astype(np.float32)       # [NKT*128, NDIR]
        wdf = (alpha[g_of_row][:, dr] * (2.0 * Sd - 7.0)
               + beta[g_of_row][:, dr])            # [NKT*128, NDIR]
        WD = np.ascontiguousarray(
            wdf.reshape(NKT, 128, NDIR).transpose(1, 0, 2)
        ).reshape(128, NKT * NDIR).astype(np.float16)
        im = {}
        # hw0 = [a2p | w chunk 0], int16 rows
        k0, nk = KT_OFF[0], KT_CHUNKS[0]
        im["hw0"] = np.ascontiguousarray(np.concatenate(
            [a2p.view(np.int16),
             np.ascontiguousarray(W[:, :nk * NWC]).view(np.int16)], axis=1))
        for ch in range(1, NCHUNK):
            k0, nk = KT_OFF[ch], KT_CHUNKS[ch]
            im[f"w{ch}"] = np.ascontiguousarray(
                W[:, k0 * NWC:(k0 + nk) * NWC]).view(np.int32)
        for ch in range(NCHUNK):
            k0, nk = KT_OFF[ch], KT_CHUNKS[ch]
            im[f"m{ch}"] = np.ascontiguousarray(np.concatenate(
                [XT[:, k0 * T:(k0 + nk) * T],
                 WD[:, k0 * NDIR:(k0 + nk) * NDIR]], axis=1))
        in_maps.append(im)
    return in_maps, corr


# ---------------------------------------------------------------- program
def build_program():
    nc = bacc.Bacc("TRN2")

    hw0_dr = nc.declare_dram_parameter(
        "hw0", [128, NPK + KT_CHUNKS[0] * NWC * 2], I16, isOutput=False)
    w_dr = {ch: nc.declare_dram_parameter(
                f"w{ch}", [128, KT_CHUNKS[ch] * NWC], I32, isOutput=False)
            for ch in range(1, NCHUNK)}
    m_dr = [nc.declare_dram_parameter(
                f"m{ch}", [128, KT_CHUNKS[ch] * (T + NDIR)], F16, isOutput=False)
            for ch in range(NCHUNK)]
    z = nc.declare_dram_parameter("z", [T, OPC], F16, isOutput=True)

    with tile.TileContext(nc) as tc, ExitStack() as ctx:
        cpool = ctx.enter_context(tc.tile_pool(name="const", bufs=1))
        opool = ctx.enter_context(tc.tile_pool(name="out", bufs=1))
        ppool = ctx.enter_context(tc.tile_pool(name="psum", bufs=1, space="PSUM"))

        # --- static tiles --------------------------------------------------
        hw0_sb = cpool.tile([128, NPK + KT_CHUNKS[0] * NWC * 2], I16, tag="hw0")
        w_sb = {ch: cpool.tile([128, KT_CHUNKS[ch] * NWC], I32, tag=f"w{ch}",
                               name=f"wsb{ch}")
                for ch in range(1, NCHUNK)}
        m_sb = [cpool.tile([128, KT_CHUNKS[ch] * (T + NDIR)], F16, tag=f"m{ch}",
                           name=f"msb{ch}")
                for ch in range(NCHUNK)]
        a2_v = hw0_sb[:, :NPK].bitcast(F16)
        w0_v = hw0_sb[:, NPK:].bitcast(I32)

        def xt_v(ch):
            return m_sb[ch][:, :KT_CHUNKS[ch] * T]

        def wd_v(ch):
            return m_sb[ch][:, KT_CHUNKS[ch] * T:]

        vt = [cpool.tile([128, KT_CHUNKS[ch] * NPK], I16, tag=f"vt{ch}",
                         name=f"vtt{ch}")
              for ch in range(NCHUNK)]
        wm = [cpool.tile([128, KT_CHUNKS[ch] * NPK], F16, tag=f"wm{ch}",
                         name=f"wmt{ch}")
              for ch in range(NCHUNK)]
        gw = cpool.tile([128, 128], F16, tag="gw")     # warm-up operand

        # each psum tile padded to a full 2KB bank (collision safety)
        psA = ppool.tile([T, 512], F32, tag="psA")
        psB = ppool.tile([T, 512], F32, tag="psB")
        psW = ppool.tile([T, 512], F32, tag="psW")     # warm-up target

        # --- DMA schedule: FIFO per queue == completion order -------------
        nc.sync.dma_start(out=hw0_sb[:], in_=hw0_dr[:])
        nc.scalar.dma_start(out=w_sb[1][:], in_=w_dr[1][:])
        nc.sync.dma_start(out=m_sb[0][:], in_=m_dr[0][:])
        nc.scalar.dma_start(out=w_sb[2][:], in_=w_dr[2][:])
        nc.scalar.dma_start(out=w_sb[3][:], in_=w_dr[3][:])
        nc.sync.dma_start(out=m_sb[1][:], in_=m_dr[1][:])
        nc.scalar.dma_start(out=m_sb[2][:], in_=m_dr[2][:])
        nc.scalar.dma_start(out=m_sb[3][:], in_=m_dr[3][:])

        # --- PE warm-up: keep the array busy through the HAM window -------
        nc.gpsimd.memset(gw[:], 0.0)
        for _ in range(N_WARM):
            nc.tensor.matmul(psW[:, :128], gw[:], gw[:], start=True, stop=True)

        # --- dequant pass 1: vt[p, r, kt, u] = W32 & (7<<3r both halves) --
        def extract(ch, r):
            nk = KT_CHUNKS[ch]
            src = w0_v if ch == 0 else w_sb[ch][:]
            m = 7 << (3 * r)
            nc.vector.tensor_scalar(
                vt[ch][:, r * nk * WPF:(r + 1) * nk * WPF].bitcast(I32),
                src, (m << 16) | m, None, ALU.bitwise_and)

        # --- dequant pass 2 (piece): wm[p,kt,o'] = vt * a2p, kts [k0,k1) --
        def scale(ch, k0, k1):
            nk = KT_CHUNKS[ch]
            npc = k1 - k0
            in0 = vt[ch][:].rearrange("p (r k u) -> p k r u", r=NR,
                                      u=WPF)[:, k0:k1]
            out = wm[ch][:].rearrange("p (k r u) -> p k r u", r=NR,
                                      u=WPF)[:, k0:k1]
            in1 = a2_v.rearrange("p (r u) -> p r u", r=NR).unsqueeze(
                1).broadcast_to([128, npc, NR, WPF])
            nc.vector.tensor_tensor(out, in0, in1, ALU.mult)

        def mmA(ch, kl):
            kt = KT_OFF[ch] + kl
            nc.tensor.matmul(
                psA[:, :NPK],
                xt_v(ch)[:, kl * T:(kl + 1) * T],
                wm[ch][:, kl * NPK:(kl + 1) * NPK],
                start=(kt == 0), stop=(kt == NKT - 1))

        def mmB(ch, kl):
            kt = KT_OFF[ch] + kl
            nc.tensor.matmul(
                psB[:, :NDIR],
                xt_v(ch)[:, kl * T:(kl + 1) * T],
                wd_v(ch)[:, kl * NDIR:(kl + 1) * NDIR],
                start=(kt == 0), stop=(kt == NKT - 1))

        out_a = opool.tile([T, NPK], F16, tag="out_a")
        out_b = opool.tile([T, NDIR], F16, tag="out_b")

        for ch in range(NCHUNK):
            nk = KT_CHUNKS[ch]
            for r in range(NR):
                extract(ch, r)
            k0 = 0
            for npc in SCALE_PIECES[ch]:
                scale(ch, k0, k0 + npc)
                for kl in range(k0, k0 + npc):
                    mmA(ch, kl)
                k0 += npc
            for kl in range(nk):
                mmB(ch, kl)
        # A finishes first (DVE-gated); B's last mega lands last
        nc.vector.tensor_copy(out_a[:], psA[:, :NPK])
        nc.sync.dma_start(out=z[:, :NPK], in_=out_a[:])
        nc.vector.tensor_copy(out_b[:], psB[:, :NDIR])
        nc.scalar.dma_start(out=z[:, NPK:], in_=out_b[:])
    nc.finalize()
    return nc


def _get_program():
    if "nc" not in _PROGRAM_CACHE:
        _PROGRAM_CACHE["nc"] = build_program()
    return _PROGRAM_CACHE["nc"]


# ---------------------------------------------------------------- entry
def kernel(**inputs):
    from concourse.bass_utils import run_bass_kernel_spmd

    in_maps, corr = _prepare(inputs)
    nc = _get_program()
    res = run_bass_kernel_spmd(nc, in_maps, list(range(NCORES)))
    out_reorder = np.asarray(inputs["out_reorder"], np.int64)
    # per-core: col j<NPK is packed field r=j//WPF (descale 8^-r, add corr);
    # col j>=NPK is direct (exact, no correction)
    rs = np.concatenate([np.repeat(8.0 ** -np.arange(NR), WPF),
                         np.ones(NDIR, np.float64)]).astype(np.float32)
    y = np.empty((T, OUT_F), np.float32)
    for c in range(NCORES):
        zc = res.results[c]["z"].astype(np.float32) * rs[None, :]
        zc[:, :NPK] += corr[:, OPC * c:OPC * c + NPK]
        y[:, OPC * c:OPC * (c + 1)] = zc
    y = y[:, out_reorder]
    return y.reshape(1, T, OUT_F).astype(np.float32)
